# revision 1
# baseline (speedup 1.0000x reference)
"""Distributed Trainium2 Bass kernel for nn_CrossAttention (B=4, L=1024,
Lc=2048, C=1024, H=16).

Sharding: 8 cores = 4 batches x 2 head-groups of 8 heads. Each core
computes its batch's q/k/v projections for its 8 heads, the attention,
and a partial output projection (row-shard of Wp). Host sums the two
partial outputs per batch and adds bp.

All matmul inputs are bf16 (fp32 PSUM accumulation); norms/softmax
internals fp32. Softmax skips the max-subtraction (logits are tiny:
l2-normalized q x k) and uses exp(S)*exp(bias) with exp(bias)
precomputed on host. The softmax division is applied per-head after
the AV matmul via a rowsum column appended to V.
"""

import os
import sys
from contextlib import ExitStack

sys.path.insert(0, "/opt/trn_rl_repo")

import numpy as np
import ml_dtypes

import concourse.bass as bass
from concourse import bacc
import concourse.mybir as mybir
import concourse.tile as tile
from concourse.bass_utils import run_bass_kernel_spmd

BF16 = ml_dtypes.bfloat16
AF = mybir.ActivationFunctionType
ALU = mybir.AluOpType
AX = mybir.AxisListType

# All ACT functions used here (Copy/Exp/Ln) live in the
# natural_log_exp_and_others table set; blank the other sets so
# insert_act_table_loads emits exactly one table load instead of
# thrashing between per-anchor sets. Indices into act_info.json are
# preserved (keys/order unchanged).
from concourse.hw_specs import get_activation_tables as _gat_orig


def _gat_one_set(arch):
    t = _gat_orig(arch)
    return {
        k: (v if k == "natural_log_exp_and_others" else set()) for k, v in t.items()
    }


bacc.get_activation_tables = _gat_one_set

# Optional experiment: let walrus dedupe/fuse redundant Ldweights
# (enable with LDWOPT=1; kept off until hardware-verified).
if os.environ.get("LDWOPT", "0") == "1":
    from concourse import bass_utils as _bu

    _orig_run_command = _bu.run_command

    def _run_command_ldwopt(argv, **kwargs):
        argv = [
            ("--enable-ldw-opt=true" if a == "--enable-ldw-opt=false" else a)
            for a in argv
        ]
        return _orig_run_command(argv, **kwargs)

    _bu.run_command = _run_command_ldwopt

B, L, LC, C, H = 4, 1024, 2048, 1024, 16
HG = 8  # heads per core
D = 64  # head dim
OC = HG * D  # 512 output channels per core
N_CORES = 8
MAX_SCALE_MUL = float(np.log(100.0))

# module-level knobs for test harness
TRACE = False
LAST_RESULT = None

_NC_CACHE = {}


def build_nc():
    f32, bf16 = mybir.dt.float32, mybir.dt.bfloat16
    nc = bacc.Bacc()

    xT = nc.declare_dram_parameter("xT", [C, L], bf16, isOutput=False)
    ctxT = nc.declare_dram_parameter("ctxT", [C, LC], bf16, isOutput=False)
    wqT = nc.declare_dram_parameter("wqT", [C, OC], bf16, isOutput=False)
    wkT = nc.declare_dram_parameter("wkT", [C, OC], bf16, isOutput=False)
    wvT = nc.declare_dram_parameter("wvT", [C, OC], bf16, isOutput=False)
    wpT = nc.declare_dram_parameter("wpT", [OC, C], bf16, isOutput=False)
    expbT = nc.declare_dram_parameter("expbT", [HG, LC, L], bf16, isOutput=False)
    hsum = nc.declare_dram_parameter("hsum", [OC, HG], bf16, isOutput=False)
    hbc = nc.declare_dram_parameter("hbc", [HG, OC], bf16, isOutput=False)
    sminv = nc.declare_dram_parameter("sminv", [HG, 1], f32, isOutput=False)
    y = nc.declare_dram_parameter("y", [L, C], f32, isOutput=True)

    KT = C // 128  # 8 contraction tiles
    OCT = OC // 128  # 4 output-channel tiles
    MT = LC // 128  # 16 context tiles
    LT = L // 128  # 8 query tiles

    with tile.TileContext(nc) as tc, ExitStack() as persist:
        # pools that live for the whole kernel
        keep = persist.enter_context(tc.tile_pool(name="keep", bufs=1))
        dma = nc.sync

        # head PAIRS stacked across the 128 partitions: proj contraction
        # becomes standard K=128 matmuls (j-th pair = heads 2j, 2j+1)
        wp_t = keep.tile([128, HG // 2, C], bf16, tag="wp")
        wp2_sb = [wp_t[:, j, :] for j in range(HG // 2)]

        kT_sb = [keep.tile([128, LC], bf16, tag=f"kT{ot}", name=f"kT{ot}") for ot in range(OCT)]
        qhat_sb = [keep.tile([128, L], bf16, tag=f"qhat{ot}", name=f"qhat{ot}") for ot in range(OCT)]
        v_sb = [keep.tile([128, HG, D + 1], bf16, tag=f"v{mt}", name=f"v{mt}") for mt in range(MT)]
        on2_sb = [keep.tile([128, L], bf16, tag=f"on2_{j}", name=f"on2_{j}") for j in range(HG // 2)]

        # ---------------- phase 1: projections + norms ----------------
        with ExitStack() as p1:
            wpool = p1.enter_context(tc.tile_pool(name="wpool", bufs=1))
            apool = p1.enter_context(tc.tile_pool(name="apool", bufs=1))
            spool = p1.enter_context(tc.tile_pool(name="spool", bufs=1))
            psA = p1.enter_context(tc.tile_pool(name="psA", bufs=3, space="PSUM"))

            # q-phase inputs first so PE can start ASAP, then k/v inputs
            wq_t = wpool.tile([128, KT, OC], bf16, tag="wq")
            wqT_r = wqT.rearrange("(t p) o -> p t o", p=128)
            for c in range(4):
                cs = slice(c * KT // 4, (c + 1) * KT // 4)
                dma.dma_start(out=wq_t[:, cs, :], in_=wqT_r[:, cs, :])
            wq_sb = [wq_t[:, kt, :] for kt in range(KT)]
            x_t = apool.tile([128, KT, L], bf16, tag="x")
            xT_r = xT.rearrange("(t p) l -> p t l", p=128)
            for c in range(4):
                cs = slice(c * KT // 4, (c + 1) * KT // 4)
                dma.dma_start(out=x_t[:, cs, :], in_=xT_r[:, cs, :])
            x_sb = [x_t[:, kt, :] for kt in range(KT)]
            wk_t = wpool.tile([128, KT, OC], bf16, tag="wk")
            dma.dma_start(out=wk_t, in_=wkT.rearrange("(t p) o -> p t o", p=128))
            wk_sb = [wk_t[:, kt, :] for kt in range(KT)]
            ctx_t = apool.tile([128, KT, LC], bf16, tag="ctx")
            dma.dma_start(out=ctx_t, in_=ctxT.rearrange("(t p) m -> p t m", p=128))
            ctx_sb = [ctx_t[:, kt, :] for kt in range(KT)]
            wv_t = wpool.tile([128, KT, OC], bf16, tag="wv")
            dma.dma_start(out=wv_t, in_=wvT.rearrange("(t p) o -> p t o", p=128))
            wv_sb = [wv_t[:, kt, :] for kt in range(KT)]
            hsum_t = wpool.tile([128, OCT, HG], bf16, tag="hsum")
            dma.dma_start(out=hsum_t, in_=hsum.rearrange("(t p) h -> p t h", p=128))
            hsum_sb = [hsum_t[:, ot, :] for ot in range(OCT)]
            hbc_sb = wpool.tile([HG, OC], bf16, tag="hbc")
            dma.dma_start(out=hbc_sb, in_=hbc[:, :])
            sminv_sb = wpool.tile([HG, 1], f32, tag="sminv")
            dma.dma_start(out=sminv_sb, in_=sminv[:, :])
            # wp is only read by the output projection (~end of kernel);
            # issue its DMA after all startup-critical loads
            dma.dma_start(out=wp_t, in_=wpT.rearrange("(j p) o -> p j o", p=128))

            # q projection: qT (f32) and q^2 (bf16) per oc-tile
            qT_sb, q2_sb = [], []
            for ot in range(OCT):
                ps = psA.tile([128, L], f32, tag="psA")
                oc_sl = slice(ot * 128, (ot + 1) * 128)
                for kt in range(KT):
                    for nch in range(L // 512):
                        nsl = slice(nch * 512, (nch + 1) * 512)
                        nc.tensor.matmul(
                            ps[:, nsl],
                            wq_sb[kt][:, oc_sl],
                            x_sb[kt][:, nsl],
                            start=(kt == 0),
                            stop=(kt == KT - 1),
                        )
                t = apool.tile([128, L], f32, tag=f"qT{ot}")
                nc.scalar.activation(t, ps, AF.Copy)
                qT_sb.append(t)
                t2 = apool.tile([128, L], bf16, tag=f"q2{ot}")
                nc.vector.tensor_mul(t2, t, t)
                q2_sb.append(t2)

            # k projection (two Lc halves per oc-tile) + k row norms
            rsk_sb = {}

            def k_proj(ot):
                oc_sl = slice(ot * 128, (ot + 1) * 128)
                n2kh = spool.tile([128, 2], f32, tag=f"n2kh{ot}")
                for half in range(2):
                    ps = psA.tile([128, 1024], f32, tag="psA")
                    for kt in range(KT):
                        for nch in range(2):
                            nsl = slice(nch * 512, (nch + 1) * 512)
                            gsl = slice(
                                half * 1024 + nch * 512, half * 1024 + (nch + 1) * 512
                            )
                            nc.tensor.matmul(
                                ps[:, nsl],
                                wk_sb[kt][:, oc_sl],
                                ctx_sb[kt][:, gsl],
                                start=(kt == 0),
                                stop=(kt == KT - 1),
                            )
                    kt_half = kT_sb[ot][:, half * 1024 : (half + 1) * 1024]
                    nc.scalar.activation(kt_half, ps, AF.Copy)
                    k2s = spool.tile([128, 1024], bf16, tag="k2s", bufs=2, name="k2s")
                    # k2s = kt*kt with fused row-sum accumulation
                    nc.vector.scalar_tensor_tensor(
                        k2s,
                        kt_half,
                        1.0,
                        kt_half,
                        op0=ALU.mult,
                        op1=ALU.mult,
                        accum_out=n2kh[:, half : half + 1],
                    )
                n2k = spool.tile([128, 1], f32, tag=f"n2k{ot}")
                nc.vector.tensor_add(n2k, n2kh[:, 0:1], n2kh[:, 1:2])
                lnk = spool.tile([128, 1], f32, tag=f"lnk{ot}")
                nc.scalar.activation(lnk, n2k, AF.Ln)
                rsk = spool.tile([128, 1], f32, tag=f"rsk{ot}", name="rsk")
                nc.scalar.activation(rsk, lnk, AF.Exp, scale=-0.5)
                rsk_sb[ot] = rsk

            # q norms: n2[h,l] -> s = sm/sqrt(n2) -> broadcast to oc rows
            with tc.tile_pool(name="psN", bufs=1, space="PSUM") as psN:
                psn2 = psN.tile([HG, L], f32, tag="psn2")
                for ot in range(OCT):
                    for nch in range(L // 512):
                        nsl = slice(nch * 512, (nch + 1) * 512)
                        nc.tensor.matmul(
                            psn2[:, nsl],
                            hsum_sb[ot],
                            q2_sb[ot][:, nsl],
                            start=(ot == 0),
                            stop=(ot == OCT - 1),
                        )
                k_proj(0)
                k_proj(1)
                t8 = spool.tile([HG, L], f32, tag="t8")
                nc.scalar.activation(t8, psn2, AF.Ln, scale=sminv_sb[:, 0:1])
            s_sb = spool.tile([HG, L], bf16, tag="s_sb")
            nc.scalar.activation(s_sb, t8, AF.Exp, scale=-0.5)
            sbc_sb = []
            for ot in range(OCT):
                ps = psA.tile([128, L], f32, tag="psA")
                for nch in range(L // 512):
                    nsl = slice(nch * 512, (nch + 1) * 512)
                    nc.tensor.matmul(
                        ps[:, nsl],
                        hbc_sb[:, ot * 128 : (ot + 1) * 128],
                        s_sb[:, nsl],
                        start=True,
                        stop=True,
                    )
                sbc = spool.tile([128, L], f32, tag=f"sbc{ot}", name="sbc")
                nc.scalar.activation(sbc, ps, AF.Copy)
                sbc_sb.append(sbc)


            k_proj(2)
            k_proj(3)

            # qhat = (qT * rsk_per_partition) * s_broadcast
            for ot in range(OCT):
                nc.vector.scalar_tensor_tensor(
                    qhat_sb[ot],
                    qT_sb[ot],
                    rsk_sb[ot][:, 0:1],
                    sbc_sb[ot],
                    op0=ALU.mult,
                    op1=ALU.mult,
                )

            # v projection into (m, head, d+1) layout with ones column
            psV = p1.enter_context(tc.tile_pool(name="psV", bufs=2, space="PSUM"))
            for mt in range(MT):
                ps = psV.tile([128, OC], f32, tag="psV")
                msl = slice(mt * 128, (mt + 1) * 128)
                for kt in range(KT):
                    nc.tensor.matmul(
                        ps,
                        ctx_sb[kt][:, msl],
                        wv_sb[kt],
                        start=(kt == 0),
                        stop=(kt == KT - 1),
                    )
                nc.scalar.activation(
                    v_sb[mt][:, :, 0:D], ps.rearrange("p (h d) -> p h d", h=HG), AF.Copy
                )
                nc.vector.memset(v_sb[mt][:, :, D : D + 1], 1.0)

        # ---------------- phase 2: attention ----------------
        with ExitStack() as p2:
            stpool = p2.enter_context(tc.tile_pool(name="stream", bufs=4))
            tpool = p2.enter_context(tc.tile_pool(name="tails", bufs=1))
            psS = p2.enter_context(tc.tile_pool(name="psS", bufs=3, space="PSUM"))
            psO = p2.enter_context(tc.tile_pool(name="psO", bufs=1, space="PSUM"))

            SKEW = 3  # AV matmuls trail S matmuls by this many m-tiles

            for hh in range(HG):
                ot, po = hh // 2, (hh % 2) * D
                pso = psO.tile([D + 1, L], f32, tag="pso")
                ebt_g = None
                ptbs = {}

                def s_stage(mt, ot=ot, po=po):
                    nonlocal ebt_g
                    msl = slice(mt * 128, (mt + 1) * 128)
                    if mt % 4 == 0:
                        ebt_g = stpool.tile(
                            [128, 4, L], bf16, tag="expb", bufs=2, name="ebt_g"
                        )
                        dma.dma_start(
                            out=ebt_g,
                            in_=expbT[hh, mt * 128 : (mt + 4) * 128, :].rearrange(
                                "(g p) l -> p g l", p=128
                            ),
                        )
                    pss = psS.tile([128, L], f32, tag="pss", name="pss")
                    for nch in range(L // 512):
                        nsl = slice(nch * 512, (nch + 1) * 512)
                        nc.tensor.matmul(
                            pss[:, nsl],
                            kT_sb[ot][po : po + D, msl],
                            qhat_sb[ot][po : po + D, nsl],
                            start=True,
                            stop=True,
                        )
                    praw = stpool.tile([128, L], bf16, tag="praw", name="praw")
                    nc.scalar.activation(praw, pss, AF.Exp)
                    ptb = stpool.tile([128, L], bf16, tag="ptb", name="ptb")
                    nc.vector.tensor_mul(ptb, praw, ebt_g[:, mt % 4, :])
                    ptbs[mt] = ptb

                def av_stage(mt, hh=hh, pso=pso):
                    ptb = ptbs.pop(mt)
                    for nch in range(L // 512):
                        nsl = slice(nch * 512, (nch + 1) * 512)
                        nc.tensor.matmul(
                            pso[:, nsl],
                            v_sb[mt][:, hh, :],
                            ptb[:, nsl],
                            start=(mt == 0),
                            stop=(mt == MT - 1),
                        )

                for mt in range(MT):
                    s_stage(mt)
                    if mt >= SKEW:
                        av_stage(mt - SKEW)
                for mt in range(MT - SKEW, MT):
                    av_stage(mt)
                # evacuate pso right away so its PSUM banks free for the
                # next head; tail math runs from SBUF.
                osb = tpool.tile([D + 1, L], f32, tag="osb", bufs=2, name="osb")
                nc.vector.tensor_copy(osb, pso)
                # recip of rowsum (partition D) via exp(-ln(.)). Reshape the
                # (1, L) row to (128, L/128) by DMA first so the two ACT ops
                # use all 128 lanes (~0.2us instead of ~1us each), then
                # reshape back to partition 0 — partition_broadcast reads
                # physical partition 0.
                rs128 = tpool.tile([128, L // 128], f32, tag="rs128", bufs=2, name="rs128")
                dma.dma_start(out=rs128, in_=osb[D : D + 1, :])
                ln128 = tpool.tile([128, L // 128], f32, tag="ln128", bufs=2, name="ln128")
                nc.scalar.activation(ln128, rs128, AF.Ln)
                rc128 = tpool.tile([128, L // 128], f32, tag="rc128", bufs=2, name="rc128")
                nc.scalar.activation(rc128, ln128, AF.Exp, scale=-1.0)
                rrec0 = tpool.tile([1, L], f32, tag="rrec0", bufs=2, name="rrec0")
                dma.dma_start(out=rrec0, in_=rc128)
                rb = tpool.tile([D, L], f32, tag="rb", bufs=2, name="rb")
                nc.gpsimd.partition_broadcast(rb, rrec0, channels=D)
                if hh % 2 == 0:
                    nc.vector.tensor_mul(on2_sb[hh // 2][0:D, :], osb[0:D, :], rb)
                else:
                    onodd = tpool.tile([D, L], bf16, tag="onodd", bufs=2, name="onodd")
                    nc.vector.tensor_mul(onodd, osb[0:D, :], rb)
                    dma.dma_start(out=on2_sb[hh // 2][D:128, :], in_=onodd)

        # ---------------- phase 3: output projection ----------------
        with ExitStack() as p3:
            ypool = p3.enter_context(tc.tile_pool(name="ypool", bufs=2))
            psY = p3.enter_context(tc.tile_pool(name="psY", bufs=2, space="PSUM"))

            for lt in range(LT):
                lsl = slice(lt * 128, (lt + 1) * 128)
                psy = psY.tile([128, C], f32, tag="psy")
                for j in range(HG // 2):
                    for nch in range(C // 512):
                        nsl = slice(nch * 512, (nch + 1) * 512)
                        nc.tensor.matmul(
                            psy[:, nsl],
                            on2_sb[j][:, lsl],
                            wp2_sb[j][:, nsl],
                            start=(j == 0),
                            stop=(j == HG // 2 - 1),
                        )
                ysb = ypool.tile([128, C], f32, tag="ysb")
                nc.scalar.activation(ysb, psy, AF.Copy)
                dma.dma_start(out=y[lsl, :], in_=ysb)

    nc.compile()
    return nc


def _get_nc():
    if "nc" not in _NC_CACHE:
        _NC_CACHE["nc"] = build_nc()
    return _NC_CACHE["nc"]


def kernel(x, context, attn_bias, Wq, Wk, Wv, Wp, bp, scale_mul):
    global LAST_RESULT
    x = np.asarray(x, dtype=np.float32)
    context = np.asarray(context, dtype=np.float32)
    attn_bias = np.asarray(attn_bias, dtype=np.float32)
    Wq = np.asarray(Wq, dtype=np.float32)
    Wk = np.asarray(Wk, dtype=np.float32)
    Wv = np.asarray(Wv, dtype=np.float32)
    Wp = np.asarray(Wp, dtype=np.float32)
    bp = np.asarray(bp, dtype=np.float32)
    scale_mul = np.asarray(scale_mul, dtype=np.float32)

    sm = np.exp(np.minimum(scale_mul, MAX_SCALE_MUL)).reshape(H)  # (H,)
    expb = np.exp(attn_bias[0])  # (H, L, Lc)

    hsum = np.zeros((OC, HG), dtype=BF16)
    hbc = np.zeros((HG, OC), dtype=BF16)
    for hh in range(HG):
        hsum[hh * D : (hh + 1) * D, hh] = 1.0
        hbc[hh, hh * D : (hh + 1) * D] = 1.0

    gshard = {}
    for g in range(2):
        rows = slice(g * OC, (g + 1) * OC)
        heads = slice(g * HG, (g + 1) * HG)
        gshard[g] = dict(
            wqT=np.ascontiguousarray(Wq[rows, :].T).astype(BF16),
            wkT=np.ascontiguousarray(Wk[rows, :].T).astype(BF16),
            wvT=np.ascontiguousarray(Wv[rows, :].T).astype(BF16),
            wpT=np.ascontiguousarray(Wp[:, rows].T).astype(BF16),
            expbT=np.ascontiguousarray(
                np.transpose(expb[heads], (0, 2, 1))
            ).astype(BF16),
            sminv=(1.0 / (sm[heads] ** 2)).reshape(HG, 1).astype(np.float32),
        )
    bshard = {}
    for b in range(B):
        bshard[b] = dict(
            xT=np.ascontiguousarray(x[b].T).astype(BF16),
            ctxT=np.ascontiguousarray(context[b].T).astype(BF16),
        )

    in_maps = []
    for core in range(N_CORES):
        b, g = core // 2, core % 2
        m = dict(hsum=hsum, hbc=hbc)
        m.update(gshard[g])
        m.update(bshard[b])
        in_maps.append(m)

    nc = _get_nc()
    res = run_bass_kernel_spmd(
        nc, in_maps, core_ids=list(range(N_CORES)), trace=TRACE
    )
    LAST_RESULT = res
    outs = [r["y"] for r in res.results]
    out = np.stack(
        [outs[2 * b] + outs[2 * b + 1] + bp[None, :] for b in range(B)]
    ).astype(np.float32)
    return out



# revision 41
# speedup vs baseline: 1.1909x; 1.1909x over previous
"""Distributed Trainium2 Bass kernel for nn_CrossAttention (B=4, L=1024,
Lc=2048, C=1024, H=16).

Sharding: 8 cores = 4 batches x 2 head-groups of 8 heads. Each core
computes its batch's q/k/v projections for its 8 heads, the attention,
and a partial output projection (row-shard of Wp). Host sums the two
partial outputs per batch and adds bp.

Precision split (driven by where errors actually land):
- The S path (q/k projections, khat/qhat, the S matmul, the bias add)
  runs fp8e4m3 with perf_mode=DoubleRow (0.5 cycles/row). Logits are
  tiny (sigma ~0.14), so fp8 noise there perturbs softmax weights by
  well under 1%.
- The V path (v projection, exp output P, the AV matmul, the output
  projection) runs fp16: elementwise errors there hit the output
  unattenuated, so fp8 would cost ~2-3% rel err.
The attention bias is added into the S PSUM tile by a 0.5*I identity
matmul (DoubleRow with stride-0 "broadcast" slot APs on both operands,
so the doubled product restores the 1x bias exactly). Softmax skips
max-subtraction; exp runs on ACT straight from PSUM to fp16; the
softmax division is applied per-head after the AV matmul via a rowsum
column appended to V (broadcast back by a K=1 ones matmul on the PE).
"""

import sys
from collections import deque
from contextlib import ExitStack

sys.path.insert(0, "/opt/trn_rl_repo")

import numpy as np
import ml_dtypes

import concourse.bass as bass
from concourse import bacc
import concourse.mybir as mybir
import concourse.tile as tile
from concourse.bass_utils import run_bass_kernel_spmd

F16 = np.float16
E4M3 = ml_dtypes.float8_e4m3
AF = mybir.ActivationFunctionType
ALU = mybir.AluOpType
DR = mybir.MatmulPerfMode.DoubleRow

# All ACT functions used here (Copy/Exp/Ln) live in the
# natural_log_exp_and_others table set; blank the other sets so
# insert_act_table_loads emits exactly one table load instead of
# thrashing between per-anchor sets.
from concourse.hw_specs import get_activation_tables as _gat_orig


def _gat_one_set(arch):
    t = _gat_orig(arch)
    return {
        k: (v if k == "natural_log_exp_and_others" else set()) for k, v in t.items()
    }


bacc.get_activation_tables = _gat_one_set

B, L, LC, C, H = 4, 1024, 2048, 1024, 16
HG = 8  # heads per core
D = 64  # head dim
OC = HG * D  # 512 output channels per core
OCT = 4  # 128-row tiles of OC
KT = 8  # 128-row contraction tiles of C
JK = 4  # DoubleRow kt-pairs
MT = 16  # 128-row context tiles
LT = 8  # query tiles
N_CORES = 8
MAX_SCALE_MUL = float(np.log(100.0))

W_SCALE = 8.0  # host pre-scale on Wq/Wk (fp8 range; q/k norm makes it free)
K_BOOST = 4.0  # khat = k * rsk * 4  (fp8 range centering)
LN_KS = float(np.log(K_BOOST))
# s carries 1/(2*K_BOOST): the extra 1/2 cancels the DoubleRow
# stride-0 double-count on the S matmul.
LN_QS = float(np.log(1.0 / (2.0 * K_BOOST)))

TRACE = False
LAST_RESULT = None

_NC_CACHE = {}


def _bc2(ap, n):
    """[P, n] AP -> [P, 2, n] with a stride-0 middle dim (DoubleRow slots)."""
    return ap.unsqueeze(1).broadcast_to([ap.shape[0], 2, n])


def build_nc():
    f32, f16, f8 = mybir.dt.float32, mybir.dt.float16, mybir.dt.float8e4
    nc = bacc.Bacc()

    xT = nc.declare_dram_parameter("xT", [C, L], f8, isOutput=False)
    ctxT = nc.declare_dram_parameter("ctxT", [C, LC], f8, isOutput=False)
    ctxLoT = nc.declare_dram_parameter("ctxLoT", [C, LC], f8, isOutput=False)
    wqT = nc.declare_dram_parameter("wqT", [C, OC], f8, isOutput=False)
    wkT = nc.declare_dram_parameter("wkT", [C, OC], f8, isOutput=False)
    wvT = nc.declare_dram_parameter("wvT", [C, OC], f8, isOutput=False)
    wvLoT = nc.declare_dram_parameter("wvLoT", [C, OC], f8, isOutput=False)
    wpT = nc.declare_dram_parameter("wpT", [OC, C], f16, isOutput=False)
    biasT = nc.declare_dram_parameter("biasT", [HG, LC, L], f8, isOutput=False)
    ident = nc.declare_dram_parameter("ident", [128, 128], f8, isOutput=False)
    hsum = nc.declare_dram_parameter("hsum", [OC, HG], f16, isOutput=False)
    hbc = nc.declare_dram_parameter("hbc", [HG, OC], f16, isOutput=False)
    sminv = nc.declare_dram_parameter("sminv", [HG, 1], f32, isOutput=False)
    y = nc.declare_dram_parameter("y", [L, C], f32, isOutput=True)

    with tile.TileContext(nc) as tc, ExitStack() as persist:
        keep = persist.enter_context(tc.tile_pool(name="keep", bufs=1))
        wpool = persist.enter_context(tc.tile_pool(name="wpool", bufs=1))
        scr = persist.enter_context(tc.tile_pool(name="scr", bufs=1))
        tails = persist.enter_context(tc.tile_pool(name="tails", bufs=1))
        stream = persist.enter_context(tc.tile_pool(name="stream", bufs=1))
        psP = persist.enter_context(tc.tile_pool(name="psP", bufs=2, space="PSUM"))
        dma = nc.sync
        dmaA = nc.scalar

        # ---- input DMAs, spread across SP/ACT HWDGE queues and
        # col-chunked: the DMA engines are serial, so startup-critical
        # transfers (wq ot0 + x -> q0; wk + ctx8 -> k0) go first.
        ident_t = keep.tile([128, 128], f8, tag="ident")
        dma.dma_start(out=ident_t, in_=ident[:, :])
        wq_t = wpool.tile([128, KT, OC], f8, tag="wq")
        wqT_r = wqT.rearrange("(t p) o -> p t o", p=128)
        dma.dma_start(out=wq_t[:, :, 0:128], in_=wqT_r[:, :, 0:128])
        x_t = wpool.tile([128, KT, L], f8, tag="x")
        xT_r = xT.rearrange("(t p) l -> p t l", p=128)
        # ctx hi/lo interleaved: slot 0 = fp8(ctx), slot 1 = fp8(residual).
        # k-proj reads hi only; the v-proj residual scheme reads both.
        ctxhl = wpool.tile([128, KT, 2, LC], f8, tag="ctxhl")
        ctxT_r = ctxT.rearrange("(t p) m -> p t m", p=128)
        ctxLoT_r = ctxLoT.rearrange("(t p) m -> p t m", p=128)
        wk_t = wpool.tile([128, KT, OC], f8, tag="wk")
        hsum_t = wpool.tile([128, OCT, HG], f16, tag="hsum")
        # Everything startup-critical rides SP in transfer-priority order
        # (the DMA engines are one serial device).
        dma.dma_start(out=x_t[:, :, 0:512], in_=xT_r[:, :, 0:512])
        dma.dma_start(out=ctxhl[:, :, 0, 0:512], in_=ctxT_r[:, :, 0:512])
        dma.dma_start(out=wk_t, in_=wkT.rearrange("(t p) o -> p t o", p=128))
        dma.dma_start(out=x_t[:, :, 512:1024], in_=xT_r[:, :, 512:1024])
        dma.dma_start(out=hsum_t, in_=hsum.rearrange("(t p) h -> p t h", p=128))
        for c in range(1, 4):
            msl = slice(c * 512, (c + 1) * 512)
            dma.dma_start(out=ctxhl[:, :, 0, msl], in_=ctxT_r[:, :, msl])
        for c in range(1, 4):
            csl = slice(c * 128, (c + 1) * 128)
            dma.dma_start(out=wq_t[:, :, csl], in_=wqT_r[:, :, csl])
        hbc_t = wpool.tile([HG, OC], f16, tag="hbc")
        dma.dma_start(out=hbc_t, in_=hbc[:, :])
        sminv_t = wpool.tile([HG, 1], f32, tag="sminv")
        dma.dma_start(out=sminv_t, in_=sminv[:, :])
        # first head's bias tiles next - they gate the first exps
        early_bias = []
        for g4 in range(2):
            _bt = stream.tile([128, 4, L], f8, tag="bias", bufs=3, name="btile")
            dma.dma_start(
                out=_bt,
                in_=biasT[0, g4 * 512 : (g4 + 1) * 512, :].rearrange(
                    "(gg p) l -> p gg l", p=128
                ),
            )
            early_bias.append(_bt)
        wv_t = wpool.tile([128, KT, OC], f8, tag="wv")
        dma.dma_start(out=wv_t, in_=wvT.rearrange("(t p) o -> p t o", p=128))
        wvlo_t = wpool.tile([128, KT, OC], f8, tag="wvlo")
        dma.dma_start(out=wvlo_t, in_=wvLoT.rearrange("(t p) o -> p t o", p=128))
        dma.dma_start(out=ctxhl[:, :, 1, 0:512], in_=ctxLoT_r[:, :, 0:512])

        khat_t = keep.tile([128, OCT, LC], f8, tag="khat")
        qhat_t = keep.tile([128, OCT, L], f8, tag="qhat")
        qT_t = keep.tile([128, OCT, L], f16, tag="qT")
        q2_t = keep.tile([128, OCT, L], f16, tag="q2")
        v_sb = [
            keep.tile([128, HG, D + 1], f16, tag=f"v_{mt}", name=f"v_{mt}")
            for mt in range(MT)
        ]
        on2_t = keep.tile([128, HG // 2, L], f16, tag="on2")
        wp_t = keep.tile([128, HG // 2, C], f16, tag="wp")
        s_sb = keep.tile([HG, L], f16, tag="s_sb")
        t8 = keep.tile([HG, L], f16, tag="t8")
        ones64 = keep.tile([1, 64], f16, tag="ones64")
        nc.vector.memset(ones64, 1.0)
        cst_qs = keep.tile([128, 1], f32, tag="cst_qs")
        nc.vector.memset(cst_qs, LN_QS)
        cst_ks = keep.tile([128, 1], f32, tag="cst_ks")
        nc.vector.memset(cst_ks, LN_KS)

        # ---- PE warm-up: tiny matmuls so the p-state ramp happens
        # during the input DMAs, not during the projections.
        psd = psP.tile([128, 512], f32, tag="psP", name="warm")
        for _ in range(26):
            nc.tensor.matmul(
                psd[0:64, 0:48], ident_t[:, 0:64], ident_t[:, 0:48],
                start=True, stop=True,
            )
        wtrash = scr.tile([64, 48], f32, tag="wtrash")
        nc.vector.tensor_copy(wtrash, psd[0:64, 0:48])
        # preload the Ln/Exp ACT table off the critical path
        wtrash2 = scr.tile([1, 1], f32, tag="wtrash2")
        nc.scalar.activation(wtrash2, cst_ks[0:1, 0:1], AF.Exp)

        # ---------------- q projections + per-head scales ----------------
        def proj_chunk(ps, w_t, moving_sl, ocsl):
            for jk in range(JK):
                nc.tensor.matmul(
                    ps,
                    w_t[:, 2 * jk : 2 * jk + 2, ocsl],
                    moving_sl(jk),
                    start=(jk == 0),
                    stop=(jk == JK - 1),
                    perf_mode=DR,
                )

        def q_proj(ot):
            ocsl = slice(ot * 128, (ot + 1) * 128)
            for nch in range(2):
                nsl = slice(nch * 512, (nch + 1) * 512)
                ps = psP.tile([128, 512], f32, tag="psP", name="qp")
                proj_chunk(
                    ps, wq_t,
                    lambda jk, nsl=nsl: x_t[:, 2 * jk : 2 * jk + 2, nsl],
                    ocsl,
                )
                # raw-q evac on ACT, q2 square on DVE (one PSUM input max)
                nc.scalar.activation(qT_t[:, ot, nsl], ps, AF.Copy)
                nc.vector.tensor_mul(
                    q2_t[:, ot, nsl], qT_t[:, ot, nsl], qT_t[:, ot, nsl]
                )

        def q_scales(hsl, ots):
            """s[h] for heads in partition-slice hsl from q2 of `ots`.

            hsum is block-diagonal over head/oc, so per-head rownorms only
            need the q2 tiles those heads live in.
            """
            nh = len(range(*hsl.indices(HG)))
            for nch in range(2):
                nsl = slice(nch * 512, (nch + 1) * 512)
                ps = psP.tile([128, 512], f32, tag="psP", name="psn")
                for ii, ot in enumerate(ots):
                    nc.tensor.matmul(
                        ps[0:nh, :],
                        hsum_t[:, ot, hsl],
                        q2_t[:, ot, nsl],
                        start=(ii == 0),
                        stop=(ii == len(ots) - 1),
                    )
                nc.scalar.activation(
                    t8[hsl, nsl], ps[0:nh, :], AF.Ln, scale=sminv_t[hsl, 0:1]
                )
                # s = sm / ||q||, fp8/DoubleRow compensation folded in
                nc.scalar.activation(
                    s_sb[hsl, nsl], t8[hsl, nsl], AF.Exp,
                    scale=-0.5, bias=cst_qs[hsl, 0:1],
                )

        # ---------------- k projection chain (per oc-tile) ----------------
        kraws = {}
        n2khs = {}

        def k_chunk(ot, gi, early=False):
            ocsl = slice(ot * 128, (ot + 1) * 128)
            if gi == 0:
                kraws[ot] = scr.tile(
                    [128, LC], f16, tag="kraw", bufs=2, name=f"kraw{ot}"
                )
                n2khs[ot] = scr.tile([128, 4], f32, tag=f"n2kh{ot}", name="n2kh")
            gsl = slice(gi * 512, (gi + 1) * 512)
            ps = psP.tile([128, 512], f32, tag="psP", name="kp")
            proj_chunk(
                ps, wk_t,
                lambda jk, gsl=gsl: ctxhl[:, 2 * jk : 2 * jk + 2, 0, gsl],
                ocsl,
            )
            if early:
                # evac on ACT so the DVE stays free for the startup chain
                nc.scalar.activation(kraws[ot][:, gsl], ps, AF.Copy)
            else:
                # during attention ACT is the bottleneck: evac on DVE
                nc.vector.tensor_copy(kraws[ot][:, gsl], ps)
            kt2 = scr.tile([128, 512], f16, tag="kt2", bufs=2, name="kt2")
            nc.vector.scalar_tensor_tensor(
                kt2, kraws[ot][:, gsl], 1.0, kraws[ot][:, gsl],
                op0=ALU.mult, op1=ALU.mult,
                accum_out=n2khs[ot][:, gi : gi + 1],
            )

        def k_rsk(ot):
            n2kh = n2khs[ot]
            t01 = scr.tile([128, 1], f32, tag="t01", bufs=2, name="t01")
            t23 = scr.tile([128, 1], f32, tag="t23", bufs=2, name="t23")
            n2k = scr.tile([128, 1], f32, tag="n2k", bufs=2, name="n2k")
            nc.vector.tensor_add(t01, n2kh[:, 0:1], n2kh[:, 1:2])
            nc.vector.tensor_add(t23, n2kh[:, 2:3], n2kh[:, 3:4])
            nc.vector.tensor_add(n2k, t01, t23)
            lnk = scr.tile([128, 1], f32, tag="lnk", bufs=2, name="lnk")
            nc.scalar.activation(lnk, n2k, AF.Ln)
            rsk = scr.tile([128, 1], f32, tag="rsk", bufs=2, name="rsk")
            # rsk = K_BOOST / ||k_row||
            nc.scalar.activation(rsk, lnk, AF.Exp, scale=-0.5, bias=cst_ks[:, 0:1])
            return rsk

        def khat_chunk(ot, gi, rsk):
            gsl = slice(gi * 512, (gi + 1) * 512)
            nc.vector.tensor_scalar_mul(
                khat_t[:, ot, gsl], kraws[ot][:, gsl], rsk[:, 0:1]
            )

        def sbc_qhat(ot, nch):
            # qhat = qT * s_broadcast(head)  (s broadcast via hbc matmul).
            # For ot0 (the early path) only s rows 0/1 exist yet - contract
            # just those two partitions so no uninitialized SBUF is read.
            nh = 2 if ot == 0 else HG
            nsl = slice(nch * 512, (nch + 1) * 512)
            ps = psP.tile([128, 512], f32, tag="psP", name="sbc")
            nc.tensor.matmul(
                ps,
                hbc_t[0:nh, ot * 128 : (ot + 1) * 128],
                s_sb[0:nh, nsl],
                start=True,
                stop=True,
            )
            nc.vector.tensor_mul(qhat_t[:, ot, nsl], qT_t[:, ot, nsl], ps)

        def v_chain(mt):
            # v = (ctx8+ctx_lo)@wv8 + ctx8@wv_lo  (drops only the lo*lo
            # term, ~0.03%) - every pass in fp8 DoubleRow.
            msl = slice(mt * 128, (mt + 1) * 128)
            ps = psP.tile([128, 512], f32, tag="psP", name="vp")
            for kt in range(KT):
                nc.tensor.matmul(
                    ps,
                    ctxhl[:, kt, :, msl],
                    _bc2(wv_t[:, kt, :], 512),
                    start=(kt == 0),
                    stop=False,
                    perf_mode=DR,
                )
            for jk in range(JK):
                nc.tensor.matmul(
                    ps,
                    ctxhl[:, 2 * jk : 2 * jk + 2, 0, msl],
                    wvlo_t[:, 2 * jk : 2 * jk + 2, :],
                    start=False,
                    stop=(jk == JK - 1),
                    perf_mode=DR,
                )
            nc.vector.tensor_copy(
                v_sb[mt][:, :, 0:D], ps.rearrange("p (h d) -> p h d", h=HG)
            )
            nc.vector.memset(v_sb[mt][:, :, D : D + 1], 1.0)

        # ---- upfront: get heads 0/1 runnable as early as possible.
        q_proj(0)
        q_scales(slice(0, 2), [0])
        for gi in range(4):
            k_chunk(0, gi, early=True)
        rsk0 = k_rsk(0)
        khat_chunk(0, 0, rsk0)
        khat_chunk(0, 1, rsk0)
        sbc_qhat(0, 0)
        sbc_qhat(0, 1)
        khat_chunk(0, 2, rsk0)
        khat_chunk(0, 3, rsk0)
        # rest of q + scales for heads 2..7 (PE-idle window while the
        # k0/khat0 DVE chain runs)
        q_proj(1)
        q_proj(2)
        q_proj(3)
        # all-heads recompute (base-0 partition APs; rows 0/1 rewrite the
        # same values the fast path produced)
        q_scales(slice(0, HG), [0, 1, 2, 3])
        for mt in range(6):
            v_chain(mt)

        # background tasks pumped into the head loop. v tiles (needed by
        # h0's own AVs) interleave with the ot1 k-chain (needed at h2).
        def k_tasks(ot):
            ts = [lambda gi=gi: k_chunk(ot, gi) for gi in range(4)]
            def k_fin():
                rsk = k_rsk(ot)
                for gi in range(4):
                    khat_chunk(ot, gi, rsk)
            ts += [k_fin, lambda: sbc_qhat(ot, 0), lambda: sbc_qhat(ot, 1)]
            return ts

        vts = [lambda mt=mt: v_chain(mt) for mt in range(6, MT)]
        k1 = k_tasks(1)
        bg = deque()
        while vts or k1:
            for _ in range(2):
                if vts:
                    bg.append(vts.pop(0))
            if k1:
                bg.append(k1.pop(0))
        for c in range(1, 4):
            def lo_dma(c=c):
                msl = slice(c * 512, (c + 1) * 512)
                dma.dma_start(out=ctxhl[:, :, 1, msl], in_=ctxLoT_r[:, :, msl])
            bg.appendleft(lo_dma)
        bg.append(lambda: dma.dma_start(
            out=wp_t, in_=wpT.rearrange("(j p) o -> p j o", p=128)))
        for ot in (2, 3):
            bg.extend(k_tasks(ot))

        def pump(n):
            for _ in range(n):
                if bg:
                    bg.popleft()()

        # ---------------- attention ----------------
        with ExitStack() as p2:
            psS = p2.enter_context(tc.tile_pool(name="psS", bufs=2, space="PSUM"))
            psO = p2.enter_context(tc.tile_pool(name="psO", bufs=1, space="PSUM"))

            SKEW = 4  # AV trails the S/exp stream by this many m-tiles
            pending_div = [None]

            def division(hh, pso):
                # per-head softmax division: out = pso[0:64]/pso[64].
                # The rowsum row (partition 64) is broadcast to partitions
                # 0..63 by a K=1 fp16 matmul against a ones-row, so the
                # whole chain stays on PE/DVE with no DMA round trips.
                osb = tails.tile([66, L], f32, tag="osb", bufs=2, name="osb")
                nc.vector.tensor_copy(osb[0 : D + 1, :], pso[0 : D + 1, :])
                rs16 = tails.tile([1, L], f16, tag="rs16", bufs=2, name="rs16")
                nc.vector.tensor_copy(rs16, osb[64:65, :])
                rbps = psO.tile([D + 2, L], f32, tag="pso", name="rbps")
                for nch in range(2):
                    nsl = slice(nch * 512, (nch + 1) * 512)
                    nc.tensor.matmul(
                        rbps[0:64, nsl],
                        ones64[:, :],
                        rs16[:, nsl],
                        start=True, stop=True,
                    )
                rrecb = tails.tile([D, L], f32, tag="rrecb", bufs=2, name="rrecb")
                nc.vector.reciprocal(rrecb, rbps[0:64, :])
                if hh % 2 == 0:
                    nc.vector.tensor_mul(
                        on2_t[0:D, hh // 2, :], osb[0:D, :], rrecb
                    )
                else:
                    onodd = tails.tile([D, L], f16, tag="onodd", bufs=2, name="onodd")
                    nc.vector.tensor_mul(onodd, osb[0:D, :], rrecb)
                    nc.gpsimd.dma_start(out=on2_t[D:128, hh // 2, :], in_=onodd)

            for hidx, hh in enumerate([0, 1, 2, 3, 4, 5, 7, 6]):
                ot, po = hh // 2, (hh % 2) * 64
                pso = psO.tile([D + 2, L], f32, tag="pso")
                ptbs = {}
                btiles = {}

                def bias_load(g4, hh=hh, btiles=btiles):
                    bt = stream.tile(
                        [128, 4, L], f8, tag="bias", bufs=3, name="btile"
                    )
                    dma.dma_start(
                        out=bt,
                        in_=biasT[hh, g4 * 512 : (g4 + 1) * 512, :].rearrange(
                            "(gg p) l -> p gg l", p=128
                        ),
                    )
                    btiles[g4] = bt

                if hidx == 0:
                    btiles[0], btiles[1] = early_bias
                else:
                    bias_load(0)
                    bias_load(1)

                def s_tile(mt, hh=hh, ot=ot, po=po, btiles=btiles, ptbs=ptbs):
                    if mt % 4 == 0 and mt + 8 < MT:
                        bias_load(mt // 4 + 2)
                    btile = btiles[mt // 4]
                    pss = psS.tile([128, L], f32, tag="pss", name="pss")
                    msl = slice(mt * 128, (mt + 1) * 128)
                    for nch in range(2):
                        nsl = slice(nch * 512, (nch + 1) * 512)
                        nc.tensor.matmul(
                            pss[:, nsl],
                            _bc2(khat_t[po : po + 64, ot, msl], 128),
                            _bc2(qhat_t[po : po + 64, ot, nsl], 512),
                            start=True,
                            stop=False,
                            perf_mode=DR,
                        )
                        # += bias via 0.5*I (stride-0 slots double it)
                        nc.tensor.matmul(
                            pss[:, nsl],
                            _bc2(ident_t[:, :], 128),
                            _bc2(btile[:, mt % 4, nsl], 512),
                            start=False,
                            stop=True,
                            perf_mode=DR,
                        )
                    ptb = stream.tile([128, L], f16, tag="ptb", bufs=5, name="ptb")
                    nc.scalar.activation(ptb, pss, AF.Exp)
                    ptbs[mt] = ptb

                def av_tile(mt, hh=hh, pso=pso, ptbs=ptbs):
                    ptb = ptbs.pop(mt)
                    for nch in range(2):
                        nsl = slice(nch * 512, (nch + 1) * 512)
                        nc.tensor.matmul(
                            pso[0 : D + 1, nsl],
                            v_sb[mt][:, hh, :],
                            ptb[:, nsl],
                            start=(mt == 0),
                            stop=(mt == MT - 1),
                        )

                for mt in range(MT):
                    s_tile(mt)
                    if mt == 0 and pending_div[0] is not None:
                        pending_div[0]()
                        pending_div[0] = None
                    if mt % 2 == 0:
                        pump(2 if hidx < 2 else 1)
                    if mt >= SKEW:
                        av_tile(mt - SKEW)
                for mt in range(MT - SKEW, MT):
                    av_tile(mt)
                pending_div[0] = lambda hh=hh, pso=pso: division(hh, pso)

            # drain any leftover background work (includes the j=2 wave)
            pump(len(bg))
            # keep the PE warm through the last division chain so the
            # final out-proj pass doesn't run at the throttled clock
            psw = psS.tile([128, L], f32, tag="pss", name="psw")
            for _ in range(7):
                nc.tensor.matmul(
                    psw[:, 0:512],
                    _bc2(ident_t[:, :], 128),
                    _bc2(qhat_t[0:128, 0, 0:512], 512),
                    start=True, stop=True, perf_mode=DR,
                )
            pending_div[0]()
            pending_div[0] = None

            # ---------------- output projection (f16) ----------------
            for lt in range(LT):
                lsl = slice(lt * 128, (lt + 1) * 128)
                psy = psS.tile([128, C], f32, tag="pss", name="psy")
                for j in range(HG // 2):
                    for nch in range(2):
                        nsl = slice(nch * 512, (nch + 1) * 512)
                        nc.tensor.matmul(
                            psy[:, nsl],
                            on2_t[:, j, lsl],
                            wp_t[:, j, nsl],
                            start=(j == 0),
                            stop=(j == HG // 2 - 1),
                        )
                ysb = tails.tile([128, C], f32, tag="ysb", bufs=2, name="ysb")
                nc.scalar.activation(ysb, psy, AF.Copy)
                dma.dma_start(out=y[lsl, :], in_=ysb)

    nc.compile()
    return nc


def _get_nc():
    if "nc" not in _NC_CACHE:
        _NC_CACHE["nc"] = build_nc()
    return _NC_CACHE["nc"]


def kernel(x, context, attn_bias, Wq, Wk, Wv, Wp, bp, scale_mul):
    global LAST_RESULT
    x = np.asarray(x, dtype=np.float32)
    context = np.asarray(context, dtype=np.float32)
    attn_bias = np.asarray(attn_bias, dtype=np.float32)
    Wq = np.asarray(Wq, dtype=np.float32)
    Wk = np.asarray(Wk, dtype=np.float32)
    Wv = np.asarray(Wv, dtype=np.float32)
    Wp = np.asarray(Wp, dtype=np.float32)
    bp = np.asarray(bp, dtype=np.float32)
    scale_mul = np.asarray(scale_mul, dtype=np.float32)

    sm = np.exp(np.minimum(scale_mul, MAX_SCALE_MUL)).reshape(H)  # (H,)

    hsum = np.zeros((OC, HG), dtype=F16)
    hbc = np.zeros((HG, OC), dtype=F16)
    for hh in range(HG):
        hsum[hh * D : (hh + 1) * D, hh] = 1.0
        hbc[hh, hh * D : (hh + 1) * D] = 1.0
    ident = (0.5 * np.eye(128, dtype=np.float32)).astype(E4M3)

    gshard = {}
    for g in range(2):
        rows = slice(g * OC, (g + 1) * OC)
        heads = slice(g * HG, (g + 1) * HG)
        wv8 = np.ascontiguousarray(Wv[rows, :].T * W_SCALE).astype(E4M3)
        gshard[g] = dict(
            wvLoT=(
                np.ascontiguousarray(Wv[rows, :].T * W_SCALE)
                - wv8.astype(np.float32)
            ).astype(E4M3),
            wqT=np.ascontiguousarray(Wq[rows, :].T * W_SCALE).astype(E4M3),
            wkT=np.ascontiguousarray(Wk[rows, :].T * W_SCALE).astype(E4M3),
            wvT=wv8,
            wpT=np.ascontiguousarray(Wp[:, rows].T / W_SCALE).astype(F16),
            biasT=np.ascontiguousarray(
                np.transpose(attn_bias[0, heads], (0, 2, 1))
            ).astype(E4M3),
            sminv=(1.0 / (sm[heads] ** 2)).reshape(HG, 1).astype(np.float32),
        )
    bshard = {}
    for b in range(B):
        ctxTb = np.ascontiguousarray(context[b].T)
        ctx8 = ctxTb.astype(E4M3)
        bshard[b] = dict(
            xT=np.ascontiguousarray(x[b].T).astype(E4M3),
            ctxT=ctx8,
            ctxLoT=(ctxTb - ctx8.astype(np.float32)).astype(E4M3),
        )

    in_maps = []
    for core in range(N_CORES):
        b, g = core // 2, core % 2
        m = dict(hsum=hsum, hbc=hbc, ident=ident)
        m.update(gshard[g])
        m.update(bshard[b])
        in_maps.append(m)

    nc = _get_nc()
    res = run_bass_kernel_spmd(
        nc, in_maps, core_ids=list(range(N_CORES)), trace=TRACE
    )
    LAST_RESULT = res
    outs = [r["y"] for r in res.results]
    out = np.stack(
        [outs[2 * b] + outs[2 * b + 1] + bp[None, :] for b in range(B)]
    ).astype(np.float32)
    return out


# revision 55
# speedup vs baseline: 1.2101x; 1.0161x over previous
"""Distributed Trainium2 Bass kernel for nn_CrossAttention (B=4, L=1024,
Lc=2048, C=1024, H=16).

Sharding: 8 cores = 4 batches x 2 head-groups of 8 heads. Each core
computes its batch's q/k/v projections for its 8 heads, the attention,
and a partial output projection (row-shard of Wp). Host sums the two
partial outputs per batch and adds bp.

Precision split (driven by where errors actually land):
- The S path (q/k projections, khat/qhat, the S matmul, the bias add)
  runs fp8e4m3 with perf_mode=DoubleRow (0.5 cycles/row). Logits are
  tiny (sigma ~0.14), so fp8 noise there perturbs softmax weights by
  well under 1%.
- The V path (v projection, exp output P, the AV matmul, the output
  projection) runs fp16: elementwise errors there hit the output
  unattenuated, so fp8 would cost ~2-3% rel err.
The attention bias is added into the S PSUM tile by a 0.5*I identity
matmul (DoubleRow with stride-0 "broadcast" slot APs on both operands,
so the doubled product restores the 1x bias exactly). Softmax skips
max-subtraction; exp runs on ACT straight from PSUM to fp16; the
softmax division is applied per-head after the AV matmul via a rowsum
column appended to V (broadcast back by a K=1 ones matmul on the PE).
"""

import sys
from collections import deque
from contextlib import ExitStack

sys.path.insert(0, "/opt/trn_rl_repo")

import numpy as np
import ml_dtypes

import concourse.bass as bass
from concourse import bacc
import concourse.mybir as mybir
import concourse.tile as tile
from concourse.bass_utils import run_bass_kernel_spmd

F16 = np.float16
E4M3 = ml_dtypes.float8_e4m3
AF = mybir.ActivationFunctionType
ALU = mybir.AluOpType
DR = mybir.MatmulPerfMode.DoubleRow

# All ACT functions used here (Copy/Exp/Ln) live in the
# natural_log_exp_and_others table set; blank the other sets so
# insert_act_table_loads emits exactly one table load instead of
# thrashing between per-anchor sets.
from concourse.hw_specs import get_activation_tables as _gat_orig


def _gat_one_set(arch):
    t = _gat_orig(arch)
    return {
        k: (v if k == "natural_log_exp_and_others" else set()) for k, v in t.items()
    }


bacc.get_activation_tables = _gat_one_set

B, L, LC, C, H = 4, 1024, 2048, 1024, 16
HG = 8  # heads per core
D = 64  # head dim
OC = HG * D  # 512 output channels per core
OCT = 4  # 128-row tiles of OC
KT = 8  # 128-row contraction tiles of C
JK = 4  # DoubleRow kt-pairs
MT = 16  # 128-row context tiles
LT = 8  # query tiles
N_CORES = 8
MAX_SCALE_MUL = float(np.log(100.0))

W_SCALE = 8.0  # host pre-scale on Wq/Wk (fp8 range; q/k norm makes it free)
K_BOOST = 4.0  # khat = k * rsk * 4  (fp8 range centering)
LN_KS = float(np.log(K_BOOST))
# s carries 1/(2*K_BOOST): the extra 1/2 cancels the DoubleRow
# stride-0 double-count on the S matmul.
LN_QS = float(np.log(1.0 / (2.0 * K_BOOST)))

TRACE = False
LAST_RESULT = None

_NC_CACHE = {}


def _bc2(ap, n):
    """[P, n] AP -> [P, 2, n] with a stride-0 middle dim (DoubleRow slots)."""
    return ap.unsqueeze(1).broadcast_to([ap.shape[0], 2, n])


def build_nc():
    f32, f16, f8 = mybir.dt.float32, mybir.dt.float16, mybir.dt.float8e4
    nc = bacc.Bacc()

    xT = nc.declare_dram_parameter("xT", [C, L], f8, isOutput=False)
    ctxT = nc.declare_dram_parameter("ctxT", [C, LC], f8, isOutput=False)
    ctxLoT = nc.declare_dram_parameter("ctxLoT", [C, LC], f8, isOutput=False)
    wqT = nc.declare_dram_parameter("wqT", [C, OC], f8, isOutput=False)
    wkT = nc.declare_dram_parameter("wkT", [C, OC], f8, isOutput=False)
    wvT = nc.declare_dram_parameter("wvT", [C, OC], f8, isOutput=False)
    wvLoT = nc.declare_dram_parameter("wvLoT", [C, OC], f8, isOutput=False)
    wpT = nc.declare_dram_parameter("wpT", [OC, C], f16, isOutput=False)
    biasT = nc.declare_dram_parameter("biasT", [HG, LC, L], f8, isOutput=False)
    ident = nc.declare_dram_parameter("ident", [128, 128], f8, isOutput=False)
    ident16 = nc.declare_dram_parameter("ident16", [128, 128], f16, isOutput=False)
    hsum = nc.declare_dram_parameter("hsum", [OC, HG], f16, isOutput=False)
    hbc = nc.declare_dram_parameter("hbc", [HG, OC], f16, isOutput=False)
    sminv = nc.declare_dram_parameter("sminv", [HG, 1], f32, isOutput=False)
    y = nc.declare_dram_parameter("y", [L, C], f16, isOutput=True)

    with tile.TileContext(nc) as tc, ExitStack() as persist:
        keep = persist.enter_context(tc.tile_pool(name="keep", bufs=1))
        wpool = persist.enter_context(tc.tile_pool(name="wpool", bufs=1))
        scr = persist.enter_context(tc.tile_pool(name="scr", bufs=1))
        tails = persist.enter_context(tc.tile_pool(name="tails", bufs=1))
        stream = persist.enter_context(tc.tile_pool(name="stream", bufs=1))
        psP = persist.enter_context(tc.tile_pool(name="psP", bufs=2, space="PSUM"))
        dma = nc.sync
        dmaA = nc.scalar

        # ---- input DMAs, spread across SP/ACT HWDGE queues and
        # col-chunked: the DMA engines are serial, so startup-critical
        # transfers (wq ot0 + x -> q0; wk + ctx8 -> k0) go first.
        ident_t = keep.tile([128, 128], f8, tag="ident")
        dma.dma_start(out=ident_t, in_=ident[:, :])
        ident16_t = keep.tile([128, 128], f16, tag="ident16")
        dma.dma_start(out=ident16_t, in_=ident16[:, :])
        wq_t = wpool.tile([128, KT, OC], f8, tag="wq")
        wqT_r = wqT.rearrange("(t p) o -> p t o", p=128)
        dma.dma_start(out=wq_t[:, :, 0:128], in_=wqT_r[:, :, 0:128])
        x_t = wpool.tile([128, KT, L], f8, tag="x")
        xT_r = xT.rearrange("(t p) l -> p t l", p=128)
        # ctx hi/lo interleaved: slot 0 = fp8(ctx), slot 1 = fp8(residual).
        # k-proj reads hi only; the v-proj residual scheme reads both.
        ctxhl = wpool.tile([128, KT, 2, LC], f8, tag="ctxhl")
        ctxT_r = ctxT.rearrange("(t p) m -> p t m", p=128)
        ctxLoT_r = ctxLoT.rearrange("(t p) m -> p t m", p=128)
        wk_t = wpool.tile([128, KT, OC], f8, tag="wk")
        hsum_t = wpool.tile([128, OCT, HG], f16, tag="hsum")
        # Everything startup-critical rides SP in transfer-priority order
        # (the DMA engines are one serial device).
        dma.dma_start(out=x_t[:, :, 0:512], in_=xT_r[:, :, 0:512])
        dma.dma_start(out=ctxhl[:, :, 0, 0:512], in_=ctxT_r[:, :, 0:512])
        dma.dma_start(out=wk_t, in_=wkT.rearrange("(t p) o -> p t o", p=128))
        dma.dma_start(out=x_t[:, :, 512:1024], in_=xT_r[:, :, 512:1024])
        dma.dma_start(out=hsum_t, in_=hsum.rearrange("(t p) h -> p t h", p=128))
        for c in range(1, 4):
            msl = slice(c * 512, (c + 1) * 512)
            dma.dma_start(out=ctxhl[:, :, 0, msl], in_=ctxT_r[:, :, msl])
        hbc_t = wpool.tile([HG, OC], f16, tag="hbc")
        dma.dma_start(out=hbc_t, in_=hbc[:, :])
        sminv_t = wpool.tile([HG, 1], f32, tag="sminv")
        dma.dma_start(out=sminv_t, in_=sminv[:, :])
        # first head's bias tiles gate the first exps - ahead of the
        # non-critical weights
        early_bias = []
        for g4 in range(2):
            _bt = stream.tile([128, 4, L], f8, tag="bias", bufs=3, name="btile")
            dma.dma_start(
                out=_bt,
                in_=biasT[0, g4 * 512 : (g4 + 1) * 512, :].rearrange(
                    "(gg p) l -> p gg l", p=128
                ),
            )
            early_bias.append(_bt)
        for c in range(1, 4):
            csl = slice(c * 128, (c + 1) * 128)
            dma.dma_start(out=wq_t[:, :, csl], in_=wqT_r[:, :, csl])
        wv_t = wpool.tile([128, KT, OC], f8, tag="wv")
        dma.dma_start(out=wv_t, in_=wvT.rearrange("(t p) o -> p t o", p=128))
        wvlo_t = wpool.tile([128, KT, OC], f8, tag="wvlo")
        dma.dma_start(out=wvlo_t, in_=wvLoT.rearrange("(t p) o -> p t o", p=128))
        dma.dma_start(out=ctxhl[:, :, 1, 0:512], in_=ctxLoT_r[:, :, 0:512])

        khat_t = keep.tile([128, OCT, LC], f8, tag="khat")
        qhat_t = keep.tile([128, OCT, L], f8, tag="qhat")
        qT_t = keep.tile([128, OCT, L], f16, tag="qT")
        q2_t = keep.tile([128, OCT, L], f16, tag="q2")
        v_sb = [
            keep.tile([128, HG, D + 1], f16, tag=f"v_{mt}", name=f"v_{mt}")
            for mt in range(MT)
        ]
        on2_t = keep.tile([128, HG // 2, L], f16, tag="on2")
        y_acc = keep.tile([128, LT, C], f16, tag="y_acc")
        wp_t = keep.tile([128, HG // 2, C], f16, tag="wp")
        s_sb = keep.tile([HG, L], f16, tag="s_sb")
        t8 = keep.tile([HG, L], f16, tag="t8")
        ones64 = keep.tile([1, 64], f16, tag="ones64")
        nc.vector.memset(ones64, 1.0)
        cst_qs = keep.tile([128, 1], f32, tag="cst_qs")
        nc.vector.memset(cst_qs, LN_QS)
        cst_ks = keep.tile([128, 1], f32, tag="cst_ks")
        nc.vector.memset(cst_ks, LN_KS)

        # ---- PE warm-up: tiny matmuls so the p-state ramp happens
        # during the input DMAs, not during the projections.
        psd = psP.tile([128, 512], f32, tag="psP", name="warm")
        for _ in range(26):
            nc.tensor.matmul(
                psd[0:64, 0:48], ident_t[:, 0:64], ident_t[:, 0:48],
                start=True, stop=True,
            )
        wtrash = scr.tile([64, 48], f32, tag="wtrash")
        nc.vector.tensor_copy(wtrash, psd[0:64, 0:48])
        # preload the Ln/Exp ACT table off the critical path
        wtrash2 = scr.tile([1, 1], f32, tag="wtrash2")
        nc.scalar.activation(wtrash2, cst_ks[0:1, 0:1], AF.Exp)

        # ---------------- q projections + per-head scales ----------------
        def proj_chunk(ps, w_t, moving_sl, ocsl):
            for jk in range(JK):
                nc.tensor.matmul(
                    ps,
                    w_t[:, 2 * jk : 2 * jk + 2, ocsl],
                    moving_sl(jk),
                    start=(jk == 0),
                    stop=(jk == JK - 1),
                    perf_mode=DR,
                )

        def q_proj(ot, early=False):
            ocsl = slice(ot * 128, (ot + 1) * 128)
            for nch in range(2):
                nsl = slice(nch * 512, (nch + 1) * 512)
                ps = psP.tile([128, 512], f32, tag="psP", name="qp")
                proj_chunk(
                    ps, wq_t,
                    lambda jk, nsl=nsl: x_t[:, 2 * jk : 2 * jk + 2, nsl],
                    ocsl,
                )
                if early:
                    # evac on ACT so the DVE stays free at startup
                    nc.scalar.activation(qT_t[:, ot, nsl], ps, AF.Copy)
                else:
                    # mid-attention ACT is the bottleneck: evac on DVE
                    nc.vector.tensor_copy(qT_t[:, ot, nsl], ps)
                nc.vector.tensor_mul(
                    q2_t[:, ot, nsl], qT_t[:, ot, nsl], qT_t[:, ot, nsl]
                )

        def q_scales(hsl, ots):
            """s[h] for heads in partition-slice hsl from q2 of `ots`.

            hsum is block-diagonal over head/oc, so per-head rownorms only
            need the q2 tiles those heads live in.
            """
            nh = len(range(*hsl.indices(HG)))
            for nch in range(2):
                nsl = slice(nch * 512, (nch + 1) * 512)
                ps = psP.tile([128, 512], f32, tag="psP", name="psn")
                for ii, ot in enumerate(ots):
                    nc.tensor.matmul(
                        ps[0:nh, :],
                        hsum_t[:, ot, hsl],
                        q2_t[:, ot, nsl],
                        start=(ii == 0),
                        stop=(ii == len(ots) - 1),
                    )
                nc.scalar.activation(
                    t8[hsl, nsl], ps[0:nh, :], AF.Ln, scale=sminv_t[hsl, 0:1]
                )
                # s = sm / ||q||, fp8/DoubleRow compensation folded in
                nc.scalar.activation(
                    s_sb[hsl, nsl], t8[hsl, nsl], AF.Exp,
                    scale=-0.5, bias=cst_qs[hsl, 0:1],
                )

        # ---------------- k projection chain (per oc-tile) ----------------
        kraws = {}
        n2khs = {}

        def k_chunk(ot, gi, early=False):
            ocsl = slice(ot * 128, (ot + 1) * 128)
            if gi == 0:
                kraws[ot] = scr.tile(
                    [128, LC], f16, tag="kraw", bufs=2, name=f"kraw{ot}"
                )
                n2khs[ot] = scr.tile([128, 4], f32, tag=f"n2kh{ot}", name="n2kh")
            gsl = slice(gi * 512, (gi + 1) * 512)
            ps = psP.tile([128, 512], f32, tag="psP", name="kp")
            proj_chunk(
                ps, wk_t,
                lambda jk, gsl=gsl: ctxhl[:, 2 * jk : 2 * jk + 2, 0, gsl],
                ocsl,
            )
            if early:
                # evac on ACT so the DVE stays free for the startup chain
                nc.scalar.activation(kraws[ot][:, gsl], ps, AF.Copy)
            else:
                # during attention ACT is the bottleneck: evac on DVE
                nc.vector.tensor_copy(kraws[ot][:, gsl], ps)
            kt2 = scr.tile([128, 512], f16, tag="kt2", bufs=2, name="kt2")
            nc.vector.scalar_tensor_tensor(
                kt2, kraws[ot][:, gsl], 1.0, kraws[ot][:, gsl],
                op0=ALU.mult, op1=ALU.mult,
                accum_out=n2khs[ot][:, gi : gi + 1],
            )

        def k_rsk(ot):
            n2kh = n2khs[ot]
            t01 = scr.tile([128, 1], f32, tag="t01", bufs=2, name="t01")
            t23 = scr.tile([128, 1], f32, tag="t23", bufs=2, name="t23")
            n2k = scr.tile([128, 1], f32, tag="n2k", bufs=2, name="n2k")
            nc.vector.tensor_add(t01, n2kh[:, 0:1], n2kh[:, 1:2])
            nc.vector.tensor_add(t23, n2kh[:, 2:3], n2kh[:, 3:4])
            nc.vector.tensor_add(n2k, t01, t23)
            lnk = scr.tile([128, 1], f32, tag="lnk", bufs=2, name="lnk")
            nc.scalar.activation(lnk, n2k, AF.Ln)
            rsk = scr.tile([128, 1], f32, tag="rsk", bufs=2, name="rsk")
            # rsk = K_BOOST / ||k_row||
            nc.scalar.activation(rsk, lnk, AF.Exp, scale=-0.5, bias=cst_ks[:, 0:1])
            return rsk

        def khat_chunk(ot, gi, rsk):
            gsl = slice(gi * 512, (gi + 1) * 512)
            nc.vector.tensor_scalar_mul(
                khat_t[:, ot, gsl], kraws[ot][:, gsl], rsk[:, 0:1]
            )

        def sbc_qhat(ot, nch):
            # qhat = qT * s_broadcast(head)  (s broadcast via hbc matmul).
            # For ot0 (the early path) only s rows 0/1 exist yet - contract
            # just those two partitions so no uninitialized SBUF is read.
            nh = 2 if ot == 0 else HG
            nsl = slice(nch * 512, (nch + 1) * 512)
            ps = psP.tile([128, 512], f32, tag="psP", name="sbc")
            nc.tensor.matmul(
                ps,
                hbc_t[0:nh, ot * 128 : (ot + 1) * 128],
                s_sb[0:nh, nsl],
                start=True,
                stop=True,
            )
            nc.vector.tensor_mul(qhat_t[:, ot, nsl], qT_t[:, ot, nsl], ps)

        def v_chain(mt):
            # v = (ctx8+ctx_lo)@wv8 + ctx8@wv_lo  (drops only the lo*lo
            # term, ~0.03%) - every pass in fp8 DoubleRow.
            msl = slice(mt * 128, (mt + 1) * 128)
            ps = psP.tile([128, 512], f32, tag="psP", name="vp")
            for kt in range(KT):
                nc.tensor.matmul(
                    ps,
                    ctxhl[:, kt, :, msl],
                    _bc2(wv_t[:, kt, :], 512),
                    start=(kt == 0),
                    stop=False,
                    perf_mode=DR,
                )
            for jk in range(JK):
                nc.tensor.matmul(
                    ps,
                    ctxhl[:, 2 * jk : 2 * jk + 2, 0, msl],
                    wvlo_t[:, 2 * jk : 2 * jk + 2, :],
                    start=False,
                    stop=(jk == JK - 1),
                    perf_mode=DR,
                )
            nc.vector.tensor_copy(
                v_sb[mt][:, :, 0:D], ps.rearrange("p (h d) -> p h d", h=HG)
            )
            nc.vector.memset(v_sb[mt][:, :, D : D + 1], 1.0)

        # ---- upfront: get heads 0/1 runnable as early as possible.
        q_proj(0, early=True)
        q_scales(slice(0, 2), [0])
        for gi in range(4):
            k_chunk(0, gi, early=True)
        rsk0 = k_rsk(0)
        khat_chunk(0, 0, rsk0)
        khat_chunk(0, 1, rsk0)
        sbc_qhat(0, 0)
        sbc_qhat(0, 1)
        khat_chunk(0, 2, rsk0)
        khat_chunk(0, 3, rsk0)
        v_chain(0)
        v_chain(1)

        # background tasks pumped into the head loop. v tiles (needed by
        # h0's own AVs) interleave with the ot1 k-chain (needed at h2).
        def k_tasks(ot):
            ts = [lambda gi=gi: k_chunk(ot, gi) for gi in range(4)]
            def k_fin():
                rsk = k_rsk(ot)
                for gi in range(4):
                    khat_chunk(ot, gi, rsk)
            ts += [k_fin, lambda: sbc_qhat(ot, 0), lambda: sbc_qhat(ot, 1)]
            return ts

        bg = deque()

        def lo_dma(c):
            msl = slice(c * 512, (c + 1) * 512)
            dma.dma_start(out=ctxhl[:, :, 1, msl], in_=ctxLoT_r[:, :, msl])

        # ordered against consumption deadlines: v_mt must be emitted
        # before AV(h0, mt) (tile mt+SKEW, with the last 4 AVs deferred to
        # h1-mt0); ctx-lo col c before v_{4c}; scales before sbc_qhat(1).
        bg.append(lambda: lo_dma(1))
        bg.append(lambda: q_proj(1))
        bg.append(lambda: q_proj(2))
        bg.append(lambda: q_proj(3))
        bg.append(lambda: v_chain(2))
        bg.append(lambda: v_chain(3))
        bg.append(lambda: lo_dma(2))
        for mt in range(4, 8):
            bg.append(lambda mt=mt: v_chain(mt))
        bg.append(lambda: lo_dma(3))
        for mt in range(8, MT):
            bg.append(lambda mt=mt: v_chain(mt))
        # all-heads scale recompute (base-0 partition APs; rows 0/1
        # rewrite the same values the fast path produced)
        bg.append(lambda: q_scales(slice(0, HG), [0, 1, 2, 3]))
        bg.extend(k_tasks(1))
        bg.append(lambda: dma.dma_start(
            out=wp_t, in_=wpT.rearrange("(j p) o -> p j o", p=128)))
        for ot in (2, 3):
            bg.extend(k_tasks(ot))

        def yw_task(j, lt):
            lsl = slice(lt * 128, (lt + 1) * 128)
            for nch in range(2):
                nsl = slice(nch * 512, (nch + 1) * 512)
                ps = psP.tile([128, 512], f32, tag="psP", name="yw")
                nc.tensor.matmul(
                    ps, on2_t[:, j, lsl], wp_t[:, j, nsl],
                    start=True, stop=True,
                )
                dst = y_acc[:, lt, nsl]
                if j == 0:
                    nc.vector.tensor_copy(dst, ps)
                else:
                    nc.vector.tensor_add(dst, dst, ps)

        def pump(n):
            for _ in range(n):
                if bg:
                    bg.popleft()()

        # ---------------- attention ----------------
        with ExitStack() as p2:
            psS = p2.enter_context(tc.tile_pool(name="psS", bufs=2, space="PSUM"))
            psO = p2.enter_context(tc.tile_pool(name="psO", bufs=1, space="PSUM"))

            SKEW = 4  # AV trails the S/exp stream by this many m-tiles
            pending_tail = [None]
            HEAD_ORDER = [0, 1, 2, 3, 4, 5, 7, 6]
            btiles = {}

            def bias_load(hidx, g4):
                hh = HEAD_ORDER[hidx]
                bt = stream.tile([128, 4, L], f8, tag="bias", bufs=3, name="btile")
                dma.dma_start(
                    out=bt,
                    in_=biasT[hh, g4 * 512 : (g4 + 1) * 512, :].rearrange(
                        "(gg p) l -> p gg l", p=128
                    ),
                )
                btiles[(hidx, g4)] = bt

            def division(hh, pso, half=None):
                # per-head softmax division: out = pso[0:64]/pso[64].
                # The rowsum row (partition 64) is broadcast to partitions
                # 0..63 by a K=1 fp16 matmul against a ones-row, so the
                # whole chain stays on PE/DVE with no DMA round trips.
                rs16 = tails.tile([1, L], f16, tag="rs16", bufs=2, name="rs16")
                nc.vector.tensor_copy(rs16, pso[64:65, :])
                osb = tails.tile([66, L], f32, tag="osb", bufs=2, name="osb")
                nc.vector.tensor_copy(osb[0:D, :], pso[0:D, :])
                rbps = psO.tile([D + 2, L], f32, tag="pso", name="rbps")
                for nch in range(2):
                    nsl = slice(nch * 512, (nch + 1) * 512)
                    nc.tensor.matmul(
                        rbps[0:64, nsl],
                        ones64[:, :],
                        rs16[:, nsl],
                        start=True, stop=True,
                    )
                rrecb = tails.tile([D, L], f32, tag="rrecb", bufs=2, name="rrecb")
                nc.vector.reciprocal(rrecb, rbps[0:64, :])
                if hh % 2 == 0:
                    nc.vector.tensor_mul(
                        on2_t[0:D, hh // 2, :], osb[0:D, :], rrecb
                    )
                else:
                    onodd = tails.tile([D, L], f16, tag="onodd", bufs=2, name="onodd")
                    nc.vector.tensor_mul(onodd, osb[0:D, :], rrecb)
                    nc.gpsimd.dma_start(out=on2_t[D:128, hh // 2, :], in_=onodd)

            btiles[(0, 0)], btiles[(0, 1)] = early_bias

            for hidx, hh in enumerate(HEAD_ORDER):
                ot, po = hh // 2, (hh % 2) * 64
                pso = psO.tile([D + 2, L], f32, tag="pso")
                ptbs = {}

                def s_tile(mt, hidx=hidx, hh=hh, ot=ot, po=po, ptbs=ptbs):
                    # stay 2 bias tiles ahead, crossing into the next head
                    if mt % 4 == 0:
                        g4n, hn = mt // 4 + 2, hidx
                        if g4n >= 4:
                            g4n, hn = g4n - 4, hidx + 1
                        if hn < HG:
                            bias_load(hn, g4n)
                    btile = btiles[(hidx, mt // 4)]
                    pss = psS.tile([128, L], f32, tag="pss", name="pss")
                    msl = slice(mt * 128, (mt + 1) * 128)
                    for nch in range(2):
                        nsl = slice(nch * 512, (nch + 1) * 512)
                        nc.tensor.matmul(
                            pss[:, nsl],
                            _bc2(khat_t[po : po + 64, ot, msl], 128),
                            _bc2(qhat_t[po : po + 64, ot, nsl], 512),
                            start=True,
                            stop=False,
                            perf_mode=DR,
                        )
                        # += bias via 0.5*I (stride-0 slots double it)
                        nc.tensor.matmul(
                            pss[:, nsl],
                            _bc2(ident_t[:, :], 128),
                            _bc2(btile[:, mt % 4, nsl], 512),
                            start=False,
                            stop=True,
                            perf_mode=DR,
                        )
                    ptb = stream.tile([128, L], f16, tag="ptb", bufs=5, name="ptb")
                    nc.scalar.activation(ptb, pss, AF.Exp)
                    ptbs[mt] = ptb

                def av_tile(mt, hh=hh, pso=pso, ptbs=ptbs):
                    ptb = ptbs.pop(mt)
                    for nch in range(2):
                        nsl = slice(nch * 512, (nch + 1) * 512)
                        nc.tensor.matmul(
                            pso[0 : D + 1, nsl],
                            v_sb[mt][:, hh, :],
                            ptb[:, nsl],
                            start=(mt == 0),
                            stop=(mt == MT - 1),
                        )

                for mt in range(MT):
                    s_tile(mt)
                    pump(2 if (hidx == 0 and mt >= 12) else 1)
                    if mt == 0 and pending_tail[0] is not None:
                        # previous head's last AVs + division, deferred into
                        # this head's stream so late v tiles / the division
                        # chain never block the exp pipeline at a boundary
                        pending_tail[0]()
                        pending_tail[0] = None
                    wv_sched = {
                        (2, 8): (0, 0, 4), (3, 0): (0, 4, 8),
                        (4, 0): (1, 0, 4), (4, 8): (1, 4, 8),
                        (6, 0): (2, 0, 4), (6, 8): (2, 4, 8),
                    }
                    if (hidx, mt) in wv_sched:
                        # out-proj wave slices for completed head pairs
                        # (appended only after the pair's deferred division)
                        j, lo, hi = wv_sched[(hidx, mt)]
                        for lt in range(lo, hi):
                            bg.append(lambda j=j, lt=lt: yw_task(j, lt))
                    if mt >= SKEW:
                        av_tile(mt - SKEW)

                def head_tail(hh=hh, pso=pso, av_tile=av_tile, last=(hidx == HG - 1)):
                    for mt in range(MT - SKEW, MT):
                        av_tile(mt)
                    division(hh, pso)

                pending_tail[0] = head_tail

            # drain any leftover background work (includes the j=2 wave)
            pump(len(bg))
            # keep the PE warm through the last division chain so the
            # final out-proj pass doesn't run at the throttled clock
            psw = psS.tile([128, L], f32, tag="pss", name="psw")
            for _ in range(7):
                nc.tensor.matmul(
                    psw[:, 0:512],
                    _bc2(ident_t[:, :], 128),
                    _bc2(qhat_t[0:128, 0, 0:512], 512),
                    start=True, stop=True, perf_mode=DR,
                )
            pending_tail[0]()
            pending_tail[0] = None

            # ---- final out-proj pass: psy = I@y_acc + on2_3@wp_3, ACT evac
            for lt in range(LT):
                lsl = slice(lt * 128, (lt + 1) * 128)
                psy = psS.tile([128, C], f32, tag="pss", name="psy")
                for nch in range(2):
                    nsl = slice(nch * 512, (nch + 1) * 512)
                    nc.tensor.matmul(
                        psy[:, nsl], ident16_t, y_acc[:, lt, nsl],
                        start=True, stop=False,
                    )
                    nc.tensor.matmul(
                        psy[:, nsl], on2_t[:, 3, lsl], wp_t[:, 3, nsl],
                        start=False, stop=True,
                    )
                ysb = tails.tile([128, C], f16, tag="ysb", bufs=3, name="ysb")
                nc.scalar.activation(ysb, psy, AF.Copy)
                dma.dma_start(out=y[lsl, :], in_=ysb)

    nc.compile()
    return nc


def _get_nc():
    if "nc" not in _NC_CACHE:
        _NC_CACHE["nc"] = build_nc()
    return _NC_CACHE["nc"]


def kernel(x, context, attn_bias, Wq, Wk, Wv, Wp, bp, scale_mul):
    global LAST_RESULT
    x = np.asarray(x, dtype=np.float32)
    context = np.asarray(context, dtype=np.float32)
    attn_bias = np.asarray(attn_bias, dtype=np.float32)
    Wq = np.asarray(Wq, dtype=np.float32)
    Wk = np.asarray(Wk, dtype=np.float32)
    Wv = np.asarray(Wv, dtype=np.float32)
    Wp = np.asarray(Wp, dtype=np.float32)
    bp = np.asarray(bp, dtype=np.float32)
    scale_mul = np.asarray(scale_mul, dtype=np.float32)

    sm = np.exp(np.minimum(scale_mul, MAX_SCALE_MUL)).reshape(H)  # (H,)

    hsum = np.zeros((OC, HG), dtype=F16)
    hbc = np.zeros((HG, OC), dtype=F16)
    for hh in range(HG):
        hsum[hh * D : (hh + 1) * D, hh] = 1.0
        hbc[hh, hh * D : (hh + 1) * D] = 1.0
    ident = (0.5 * np.eye(128, dtype=np.float32)).astype(E4M3)
    ident16 = np.eye(128, dtype=np.float32).astype(F16)

    gshard = {}
    for g in range(2):
        rows = slice(g * OC, (g + 1) * OC)
        heads = slice(g * HG, (g + 1) * HG)
        wv8 = np.ascontiguousarray(Wv[rows, :].T * W_SCALE).astype(E4M3)
        gshard[g] = dict(
            wvLoT=(
                np.ascontiguousarray(Wv[rows, :].T * W_SCALE)
                - wv8.astype(np.float32)
            ).astype(E4M3),
            wqT=np.ascontiguousarray(Wq[rows, :].T * W_SCALE).astype(E4M3),
            wkT=np.ascontiguousarray(Wk[rows, :].T * W_SCALE).astype(E4M3),
            wvT=wv8,
            wpT=np.ascontiguousarray(Wp[:, rows].T / W_SCALE).astype(F16),
            biasT=np.ascontiguousarray(
                np.transpose(attn_bias[0, heads], (0, 2, 1))
            ).astype(E4M3),
            sminv=(1.0 / (sm[heads] ** 2)).reshape(HG, 1).astype(np.float32),
        )
    bshard = {}
    for b in range(B):
        ctxTb = np.ascontiguousarray(context[b].T)
        ctx8 = ctxTb.astype(E4M3)
        bshard[b] = dict(
            xT=np.ascontiguousarray(x[b].T).astype(E4M3),
            ctxT=ctx8,
            ctxLoT=(ctxTb - ctx8.astype(np.float32)).astype(E4M3),
        )

    in_maps = []
    for core in range(N_CORES):
        b, g = core // 2, core % 2
        m = dict(hsum=hsum, hbc=hbc, ident=ident, ident16=ident16)
        m.update(gshard[g])
        m.update(bshard[b])
        in_maps.append(m)

    nc = _get_nc()
    res = run_bass_kernel_spmd(
        nc, in_maps, core_ids=list(range(N_CORES)), trace=TRACE
    )
    LAST_RESULT = res
    outs = [np.asarray(r["y"], dtype=np.float32) for r in res.results]
    out = np.stack(
        [outs[2 * b] + outs[2 * b + 1] + bp[None, :] for b in range(B)]
    ).astype(np.float32)
    return out
